# revision 24
# baseline (speedup 1.0000x reference)
"""CoLA linear kernel for Trainium2: y = x @ kron(U, V) + b.

Math: per token t (16384 of them), with X_t = x[t].reshape(64, 64),
    y[t] = flatten(U^T @ X_t @ V) + b     (row-major flatten, d' = 64*k + l)

Distribution: pure data parallel over tokens, 2048 per NeuronCore x 8 cores;
U, V are tiny and replicated; b is added on the host (zeros in practice).

v3 design — the kernel is memory-bound, so the layout is chosen to minimize
HBM bytes and DMA descriptor overhead:

  - fp16 device I/O: x is converted fp16 and pre-permuted on the host into
    the exact SBUF layout the matmuls need; y is written fp16 in the PSUM
    layout and un-permuted on the host.  Device HBM traffic halves to
    16 MiB in + 16 MiB out per core, and every DMA is [p=128, 8 KiB
    contiguous per partition] -- maximal descriptors, no on-chip permutes.
  - x SBUF tile (128 tokens): [p=(a2,i), f=(c,m,j)], token = o*128 + c*64
    + 2*m + a2, d = 64*i + j.
  - Stage 1 (contract i): per m-pair mp, lhsT = x slice [p=(a2,i), f=(g,j)]
    (stationary, FWL since fp16 128-col), rhs = UU = kron(I2, U) ->
    W [p=(g,j), f=(mp8,a2,k)] in a 2-bank PSUM tile (8 matmuls, N=128).
  - DVE evacuates W to SBUF fp16 (one FD=1024 copy per 2-bank tile).
  - Stage 2 (contract j): lhsT = VV = kron(I2, V) (stationary const),
    rhs = wt halves moving (N=512) -> Y [p=(g,l), f=(s,mp4,a2,k)].
  - ACT evacuates Y to SBUF fp16; output DMA on the ACT HWDGE ring.
  - fp32 PSUM accumulation throughout; measured rel err ~5e-4.
"""

import os

import numpy as np

import concourse.bacc as bacc
import concourse.bass as bass
import concourse.mybir as mybir
import concourse.tile as tile
from concourse.bass_utils import run_bass_kernel_spmd

N_CORES = 8
B, S, D = 4, 4096, 4096
T = B * S                  # 16384 tokens
TPC = T // N_CORES         # 2048 tokens per core
TOK_PER_TILE = 128         # tokens handled per DMA tile
N_TILES = TPC // TOK_PER_TILE  # 16

F32 = mybir.dt.float32
F16 = mybir.dt.float16

LAST_RESULTS = None        # test harness can inspect exec_time_ns etc.

_CACHE: dict = {}


def _build_nc(tpc: int = TPC) -> bass.Bass:
    n_tiles = tpc // TOK_PER_TILE
    nc = bacc.Bacc()

    x = nc.dram_tensor("x", [128, n_tiles * 4096], F16, kind="ExternalInput")
    uu = nc.dram_tensor("uu", [128, 128], F16, kind="ExternalInput")
    vv = nc.dram_tensor("vv", [128, 128], F16, kind="ExternalInput")
    y = nc.dram_tensor("y", [128, n_tiles * 4096], F16, kind="ExternalOutput")

    xv = x[:].rearrange("p (o f) -> o p f", f=4096)
    yv = y[:].rearrange("p (o f) -> o p f", f=4096)

    with tile.TileContext(nc) as tc:
        with (
            tc.tile_pool(name="consts", bufs=1) as cpool,
            tc.tile_pool(name="xh", bufs=4) as xh_pool,
            tc.tile_pool(name="wt", bufs=6) as wt_pool,
            tc.tile_pool(name="yo", bufs=4) as y_pool,
            tc.tile_pool(name="pw", bufs=2, space="PSUM") as pw_pool,
            tc.tile_pool(name="py", bufs=4, space="PSUM") as py_pool,
        ):
            # consts go on the ACT ring so the Sync engine's first issue is
            # the tile-0 input DMA (each HWDGE issue costs ~0.7us of NX time
            # and they serialize per engine).
            uu_sb = cpool.tile([128, 128], F16)
            nc.scalar.dma_start(out=uu_sb[:], in_=uu[:])
            vv_sb = cpool.tile([128, 128], F16)
            nc.scalar.dma_start(out=vv_sb[:], in_=vv[:])

            # ~2.5us of dummy matmuls on an uninitialized tile (no DMA
            # dependency, so they start right after the entry barrier): keeps
            # the PE busy from t~6.5us so the HAM clock-gate lifts to 2.4 GHz
            # by the time the real matmul stream is underway.
            warm_sb = cpool.tile([128, 128], F16)
            nc.gpsimd.memset(warm_sb[:], 0.0)
            pwarm = pw_pool.tile([128, 1024], F32, tag="pw")
            for w in range(10):
                nc.tensor.matmul(
                    pwarm[:, (w % 8) * 128:(w % 8 + 1) * 128],
                    warm_sb[:],
                    warm_sb[:],
                    start=True,
                    stop=True,
                )

            for o in range(n_tiles):
                # one 1-MiB input DMA, 8 KiB contiguous per partition; the
                # first two tiles stream in as quarter-DMAs so group (0,0)'s
                # matmuls start after 256 KiB instead of 1 MiB.
                xh = xh_pool.tile([128, 4096], F16)
                if o < 2:
                    for q in range(4):
                        sl = slice(q * 1024, (q + 1) * 1024)
                        nc.sync.dma_start(out=xh[:, sl], in_=xv[o][:, sl])
                else:
                    nc.sync.dma_start(out=xh[:], in_=xv[o])

                yt = y_pool.tile([128, 4096], F16)
                for c in range(2):
                    for h in range(2):
                        # stage 1: W[p=(g,j), f=(mp8,a2,k)] for 32 tokens
                        pw = pw_pool.tile([128, 1024], F32)
                        for mp8 in range(8):
                            mp = h * 8 + mp8
                            nc.tensor.matmul(
                                pw[:, mp8 * 128:(mp8 + 1) * 128],
                                xh[:, c * 2048 + mp * 128:
                                   c * 2048 + (mp + 1) * 128],
                                uu_sb[:],
                                start=True,
                                stop=True,
                            )
                        wt = wt_pool.tile([128, 1024], F16)
                        nc.vector.tensor_copy(out=wt[:], in_=pw[:])

                        # stage 2: Y[p=(g,l), f=(s,mp4,a2,k)], VV stationary;
                        # one PSUM bank per matmul so ACT evacuates each half
                        # as soon as its matmul drains.  Four 1-bank py slots
                        # (not two 2-bank ones) -- the extra pipeline depth is
                        # worth more than the per-op overhead of FD=512 reads.
                        for s in range(2):
                            py = py_pool.tile([128, 512], F32)
                            nc.tensor.matmul(
                                py[:],
                                vv_sb[:],
                                wt[:, s * 512:(s + 1) * 512],
                                start=True,
                                stop=True,
                            )
                            off = (c * 2 + h) * 1024 + s * 512
                            # ~6% of Y evacuations go to DVE to balance the
                            # ACT/DVE totals (ACT otherwise runs ~90us vs
                            # DVE ~78us per core).
                            if o % 2 == 1 and c == 0 and h == 0 and s == 0:
                                nc.vector.tensor_copy(
                                    out=yt[:, off:off + 512], in_=py[:]
                                )
                            else:
                                nc.scalar.copy(out=yt[:, off:off + 512], in_=py[:])
                # output goes out on the SWDGE ring from the otherwise-idle
                # GpSimd engine: the SP HWDGE ring keeps the input stream,
                # and ACT does evacuation only.  The last two tiles go out
                # as per-group quarter-DMAs (alternating rings) so the tail
                # bytes hit HBM right after each compute group instead of
                # waiting for the whole tile.
                yvo = yv[o]
                if o >= n_tiles - 2:
                    for q in range(4):
                        sl = slice(q * 1024, (q + 1) * 1024)
                        eng = nc.gpsimd if q % 2 == 0 else nc.scalar
                        eng.dma_start(out=yvo[:, sl], in_=yt[:, sl])
                else:
                    nc.gpsimd.dma_start(out=yvo, in_=yt[:])

    nc.finalize()
    return nc


def _get_nc() -> bass.Bass:
    if "nc" not in _CACHE:
        _CACHE["nc"] = _build_nc()
    return _CACHE["nc"]


def kernel(x: np.ndarray, U: np.ndarray, V: np.ndarray, b: np.ndarray) -> np.ndarray:
    global LAST_RESULTS
    assert x.shape == (B, S, D) and U.shape == (64, 64) and V.shape == (64, 64)

    nc = _get_nc()

    eye2 = np.eye(2, dtype=np.float32)
    uu = np.kron(eye2, np.asarray(U, dtype=np.float32)).astype(np.float16)
    vv = np.kron(eye2, np.asarray(V, dtype=np.float32)).astype(np.float16)

    xf = np.asarray(x, dtype=np.float32).reshape(T, D)
    in_maps = []
    for c in range(N_CORES):
        # token tau = o*128 + c2*64 + 2*m + a2, d = 64*i + j:
        # axes (o, c2, m, a2, i, j) -> [p=(a2,i), f=(o, c2, m, j)]
        xdev = (
            xf[c * TPC:(c + 1) * TPC]
            .reshape(N_TILES, 2, 32, 2, 64, 64)
            .astype(np.float16)
            .transpose(3, 4, 0, 1, 2, 5)
            .reshape(128, N_TILES * 4096)
        )
        in_maps.append({"x": np.ascontiguousarray(xdev), "uu": uu, "vv": vv})

    res = run_bass_kernel_spmd(
        nc,
        in_maps,
        core_ids=list(range(N_CORES)),
        trace=bool(os.environ.get("BASS_TRACE")),
    )
    LAST_RESULTS = res

    out = np.empty((T, D), dtype=np.float32)
    for c in range(N_CORES):
        # y_dev axes (g, l | o | c2, h, s, mp4, a2, k)
        #   token tau = o*128 + c2*64 + 32*h + 16*s + 4*mp4 + 2*g + a2
        #   d' = 64*k + l
        ydev = res.results[c]["y"].reshape(2, 64, N_TILES, 2, 2, 2, 4, 2, 64)
        out[c * TPC:(c + 1) * TPC] = (
            ydev.transpose(2, 3, 4, 5, 6, 0, 7, 8, 1).reshape(TPC, D)
        )

    if np.any(np.asarray(b) != 0):
        out += np.asarray(b, dtype=np.float32)
    return out.reshape(B, S, D)


# revision 26
# speedup vs baseline: 1.0199x; 1.0199x over previous
"""CoLA linear kernel for Trainium2: y = x @ kron(U, V) + b.

Math: per token t (16384 of them), with X_t = x[t].reshape(64, 64),
    y[t] = flatten(U^T @ X_t @ V) + b     (row-major flatten, d' = 64*k + l)

Distribution: pure data parallel over tokens, 2048 per NeuronCore x 8 cores;
U, V are tiny and replicated; b is added on the host (zeros in practice).

v3 design — the kernel is memory-bound, so the layout is chosen to minimize
HBM bytes and DMA descriptor overhead:

  - fp16 device I/O: x is converted fp16 and pre-permuted on the host into
    the exact SBUF layout the matmuls need; y is written fp16 in the PSUM
    layout and un-permuted on the host.  Device HBM traffic halves to
    16 MiB in + 16 MiB out per core, and every DMA is [p=128, 8 KiB
    contiguous per partition] -- maximal descriptors, no on-chip permutes.
  - x SBUF tile (128 tokens): [p=(a2,i), f=(c,m,j)], token = o*128 + c*64
    + 2*m + a2, d = 64*i + j.
  - Stage 1 (contract i): per m-pair mp, lhsT = x slice [p=(a2,i), f=(g,j)]
    (stationary, FWL since fp16 128-col), rhs = UU = kron(I2, U) ->
    W [p=(g,j), f=(mp8,a2,k)] in a 2-bank PSUM tile (8 matmuls, N=128).
  - DVE evacuates W to SBUF fp16 (one FD=1024 copy per 2-bank tile).
  - Stage 2 (contract j): lhsT = VV = kron(I2, V) (stationary const),
    rhs = wt halves moving (N=512) -> Y [p=(g,l), f=(s,mp4,a2,k)].
  - ACT evacuates Y to SBUF fp16; output DMA on the ACT HWDGE ring.
  - fp32 PSUM accumulation throughout; measured rel err ~5e-4.
"""

import os

import numpy as np

import concourse.bacc as bacc
import concourse.bass as bass
import concourse.mybir as mybir
import concourse.tile as tile
from concourse.bass_utils import run_bass_kernel_spmd

N_CORES = 8
B, S, D = 4, 4096, 4096
T = B * S                  # 16384 tokens
TPC = T // N_CORES         # 2048 tokens per core
TOK_PER_TILE = 128         # tokens handled per DMA tile
N_TILES = TPC // TOK_PER_TILE  # 16

F32 = mybir.dt.float32
F16 = mybir.dt.float16

LAST_RESULTS = None        # test harness can inspect exec_time_ns etc.

_CACHE: dict = {}


def _build_nc(tpc: int = TPC) -> bass.Bass:
    n_tiles = tpc // TOK_PER_TILE
    nc = bacc.Bacc()

    x = nc.dram_tensor("x", [128, n_tiles * 4096], F16, kind="ExternalInput")
    uu = nc.dram_tensor("uu", [128, 128], F16, kind="ExternalInput")
    vv = nc.dram_tensor("vv", [128, 128], F16, kind="ExternalInput")
    y = nc.dram_tensor("y", [128, n_tiles * 4096], F16, kind="ExternalOutput")

    xv = x[:].rearrange("p (o f) -> o p f", f=4096)
    yv = y[:].rearrange("p (o f) -> o p f", f=4096)

    with tile.TileContext(nc) as tc:
        with (
            tc.tile_pool(name="consts", bufs=1) as cpool,
            tc.tile_pool(name="xh", bufs=4) as xh_pool,
            tc.tile_pool(name="wt", bufs=6) as wt_pool,
            tc.tile_pool(name="yo", bufs=4) as y_pool,
            tc.tile_pool(name="pw", bufs=2, space="PSUM") as pw_pool,
            tc.tile_pool(name="py", bufs=4, space="PSUM") as py_pool,
        ):
            # consts go on the ACT ring so the Sync engine's first issue is
            # the tile-0 input DMA (each HWDGE issue costs ~0.7us of NX time
            # and they serialize per engine).
            uu_sb = cpool.tile([128, 128], F16)
            nc.scalar.dma_start(out=uu_sb[:], in_=uu[:])
            vv_sb = cpool.tile([128, 128], F16)
            nc.scalar.dma_start(out=vv_sb[:], in_=vv[:])

            # ~2.5us of dummy matmuls on an uninitialized tile (no DMA
            # dependency, so they start right after the entry barrier): keeps
            # the PE busy from t~6.5us so the HAM clock-gate lifts to 2.4 GHz
            # by the time the real matmul stream is underway.
            warm_sb = cpool.tile([128, 128], F16)
            nc.gpsimd.memset(warm_sb[:], 0.0)
            pwarm = pw_pool.tile([128, 1024], F32, tag="pw")
            for w in range(24):
                nc.tensor.matmul(
                    pwarm[:, (w % 8) * 128:(w % 8 + 1) * 128],
                    warm_sb[:],
                    warm_sb[:],
                    start=True,
                    stop=True,
                )

            for o in range(n_tiles):
                # one 1-MiB input DMA, 8 KiB contiguous per partition; the
                # first two tiles stream in as quarter-DMAs so group (0,0)'s
                # matmuls start after 256 KiB instead of 1 MiB.
                xh = xh_pool.tile([128, 4096], F16)
                if o < 2:
                    for q in range(4):
                        sl = slice(q * 1024, (q + 1) * 1024)
                        nc.sync.dma_start(out=xh[:, sl], in_=xv[o][:, sl])
                else:
                    nc.sync.dma_start(out=xh[:], in_=xv[o])

                yt = y_pool.tile([128, 4096], F16)
                for c in range(2):
                    for h in range(2):
                        # stage 1: W[p=(g,j), f=(mp8,a2,k)] for 32 tokens
                        pw = pw_pool.tile([128, 1024], F32)
                        for mp8 in range(8):
                            mp = h * 8 + mp8
                            nc.tensor.matmul(
                                pw[:, mp8 * 128:(mp8 + 1) * 128],
                                xh[:, c * 2048 + mp * 128:
                                   c * 2048 + (mp + 1) * 128],
                                uu_sb[:],
                                start=True,
                                stop=True,
                            )
                        wt = wt_pool.tile([128, 1024], F16)
                        nc.vector.tensor_copy(out=wt[:], in_=pw[:])

                        # stage 2: Y[p=(g,l), f=(s,mp4,a2,k)], VV stationary;
                        # one PSUM bank per matmul so ACT evacuates each half
                        # as soon as its matmul drains.  Four 1-bank py slots
                        # (not two 2-bank ones) -- the extra pipeline depth is
                        # worth more than the per-op overhead of FD=512 reads.
                        for s in range(2):
                            py = py_pool.tile([128, 512], F32)
                            nc.tensor.matmul(
                                py[:],
                                vv_sb[:],
                                wt[:, s * 512:(s + 1) * 512],
                                start=True,
                                stop=True,
                            )
                            off = (c * 2 + h) * 1024 + s * 512
                            # ~6% of Y evacuations go to DVE to balance the
                            # ACT/DVE totals (ACT otherwise runs ~90us vs
                            # DVE ~78us per core).
                            if o % 2 == 1 and c == 0 and h == 0 and s == 0:
                                nc.vector.tensor_copy(
                                    out=yt[:, off:off + 512], in_=py[:]
                                )
                            else:
                                nc.scalar.copy(out=yt[:, off:off + 512], in_=py[:])
                # output goes out on the SWDGE ring from the otherwise-idle
                # GpSimd engine: the SP HWDGE ring keeps the input stream,
                # and ACT does evacuation only.  The last two tiles go out
                # as per-group quarter-DMAs (alternating rings) so the tail
                # bytes hit HBM right after each compute group instead of
                # waiting for the whole tile.
                yvo = yv[o]
                if o >= n_tiles - 2:
                    for q in range(4):
                        sl = slice(q * 1024, (q + 1) * 1024)
                        eng = nc.gpsimd if q % 2 == 0 else nc.scalar
                        eng.dma_start(out=yvo[:, sl], in_=yt[:, sl])
                elif o % 2 == 0:
                    nc.gpsimd.dma_start(out=yvo, in_=yt[:])
                else:
                    nc.scalar.dma_start(out=yvo, in_=yt[:])

    nc.finalize()
    return nc


def _get_nc() -> bass.Bass:
    if "nc" not in _CACHE:
        _CACHE["nc"] = _build_nc()
    return _CACHE["nc"]


def kernel(x: np.ndarray, U: np.ndarray, V: np.ndarray, b: np.ndarray) -> np.ndarray:
    global LAST_RESULTS
    assert x.shape == (B, S, D) and U.shape == (64, 64) and V.shape == (64, 64)

    nc = _get_nc()

    eye2 = np.eye(2, dtype=np.float32)
    uu = np.kron(eye2, np.asarray(U, dtype=np.float32)).astype(np.float16)
    vv = np.kron(eye2, np.asarray(V, dtype=np.float32)).astype(np.float16)

    xf = np.asarray(x, dtype=np.float32).reshape(T, D)
    in_maps = []
    for c in range(N_CORES):
        # token tau = o*128 + c2*64 + 2*m + a2, d = 64*i + j:
        # axes (o, c2, m, a2, i, j) -> [p=(a2,i), f=(o, c2, m, j)]
        xdev = (
            xf[c * TPC:(c + 1) * TPC]
            .reshape(N_TILES, 2, 32, 2, 64, 64)
            .astype(np.float16)
            .transpose(3, 4, 0, 1, 2, 5)
            .reshape(128, N_TILES * 4096)
        )
        in_maps.append({"x": np.ascontiguousarray(xdev), "uu": uu, "vv": vv})

    res = run_bass_kernel_spmd(
        nc,
        in_maps,
        core_ids=list(range(N_CORES)),
        trace=bool(os.environ.get("BASS_TRACE")),
    )
    LAST_RESULTS = res

    out = np.empty((T, D), dtype=np.float32)
    for c in range(N_CORES):
        # y_dev axes (g, l | o | c2, h, s, mp4, a2, k)
        #   token tau = o*128 + c2*64 + 32*h + 16*s + 4*mp4 + 2*g + a2
        #   d' = 64*k + l
        ydev = res.results[c]["y"].reshape(2, 64, N_TILES, 2, 2, 2, 4, 2, 64)
        out[c * TPC:(c + 1) * TPC] = (
            ydev.transpose(2, 3, 4, 5, 6, 0, 7, 8, 1).reshape(TPC, D)
        )

    if np.any(np.asarray(b) != 0):
        out += np.asarray(b, dtype=np.float32)
    return out.reshape(B, S, D)


# revision 28
# speedup vs baseline: 1.0572x; 1.0366x over previous
"""CoLA linear kernel for Trainium2: y = x @ kron(U, V) + b.

Math: per token t (16384 of them), with X_t = x[t].reshape(64, 64),
    y[t] = flatten(U^T @ X_t @ V) + b     (row-major flatten, d' = 64*k + l)

Distribution: pure data parallel over tokens, 2048 per NeuronCore x 8 cores;
U, V are tiny and replicated; b is added on the host (zeros in practice).

The kernel is memory-bound, so the layout minimizes HBM bytes and DMA
descriptor overhead; the steady state is paced by PSUM evacuation:

  - fp16 device I/O: x is converted fp16 and pre-permuted on the host into
    the exact SBUF layout the matmuls need; y is written fp16 in the PSUM
    layout and un-permuted on the host.  Device HBM traffic halves to
    16 MiB in + 16 MiB out per core, and every DMA is [p=128, 8 KiB
    contiguous per partition] -- maximal descriptors, no on-chip permutes.
    Measured combined r+w DMA rate mid-run: ~425 GB/s per core.
  - x SBUF tile (128 tokens): [p=(a2,i), f=(c,m,j)], token = o*128 + c*64
    + 2*m + a2, d = 64*i + j.
  - Stage 1 (contract i): per m-pair mp, lhsT = x slice [p=(a2,i), f=(g,j)]
    (stationary, FWL since fp16 128-col), rhs = UU = kron(I2, U) ->
    W [p=(g,j), f=(mp8,a2,k)] in a 2-bank PSUM tile (8 matmuls, N=128).
  - DVE evacuates W to SBUF fp16 (one FD=1024 copy per 2-bank tile).
  - Stage 2 (contract j): lhsT = VV = kron(I2, V) (stationary const),
    rhs = wt halves moving (N=512) -> Y [p=(g,l), f=(s,mp4,a2,k)] in
    1-bank PSUM tiles (4 slots: pipeline depth beats per-op overhead).
  - ACT evacuates Y to SBUF fp16 (plus ~6% on DVE to balance engine
    totals); output DMAs ride the SWDGE ring from the idle GpSimd engine.
  - PSUM->SBUF evacuation (1x mode, two engines) is the steady-state
    pace at ~5.2us/128-token tile; a 24-matmul warmup burst lifts the PE
    HAM clock-gate before the real stream, and the first/last tiles use
    quarter-DMAs to shorten pipeline ramp and drain.
  - fp32 PSUM accumulation throughout; measured rel err ~4.7e-4.
"""

import os

import numpy as np

import concourse.bacc as bacc
import concourse.bass as bass
import concourse.mybir as mybir
import concourse.tile as tile
from concourse.bass_utils import run_bass_kernel_spmd

N_CORES = 8
B, S, D = 4, 4096, 4096
T = B * S                  # 16384 tokens
TPC = T // N_CORES         # 2048 tokens per core
TOK_PER_TILE = 128         # tokens handled per DMA tile
N_TILES = TPC // TOK_PER_TILE  # 16

F32 = mybir.dt.float32
F16 = mybir.dt.float16

LAST_RESULTS = None        # test harness can inspect exec_time_ns etc.

_CACHE: dict = {}


def _build_nc(tpc: int = TPC) -> bass.Bass:
    n_tiles = tpc // TOK_PER_TILE
    nc = bacc.Bacc()

    x = nc.dram_tensor("x", [128, n_tiles * 4096], F16, kind="ExternalInput")
    uu = nc.dram_tensor("uu", [128, 128], F16, kind="ExternalInput")
    vv = nc.dram_tensor("vv", [128, 128], F16, kind="ExternalInput")
    y = nc.dram_tensor("y", [128, n_tiles * 4096], F16, kind="ExternalOutput")

    xv = x[:].rearrange("p (o f) -> o p f", f=4096)
    yv = y[:].rearrange("p (o f) -> o p f", f=4096)

    with tile.TileContext(nc) as tc:
        with (
            tc.tile_pool(name="consts", bufs=1) as cpool,
            tc.tile_pool(name="xh", bufs=4) as xh_pool,
            tc.tile_pool(name="wt", bufs=6) as wt_pool,
            tc.tile_pool(name="yo", bufs=4) as y_pool,
            tc.tile_pool(name="pw", bufs=2, space="PSUM") as pw_pool,
            tc.tile_pool(name="py", bufs=4, space="PSUM") as py_pool,
        ):
            # consts go on the ACT ring so the Sync engine's first issue is
            # the tile-0 input DMA (each HWDGE issue costs ~0.7us of NX time
            # and they serialize per engine).
            uu_sb = cpool.tile([128, 128], F16)
            nc.scalar.dma_start(out=uu_sb[:], in_=uu[:])
            vv_sb = cpool.tile([128, 128], F16)
            nc.scalar.dma_start(out=vv_sb[:], in_=vv[:])

            # ~2.5us of dummy matmuls on an uninitialized tile (no DMA
            # dependency, so they start right after the entry barrier): keeps
            # the PE busy from t~6.5us so the HAM clock-gate lifts to 2.4 GHz
            # by the time the real matmul stream is underway.
            warm_sb = cpool.tile([128, 128], F16)
            nc.gpsimd.memset(warm_sb[:], 0.0)
            pwarm = pw_pool.tile([128, 1024], F32, tag="pw")
            for w in range(24):
                nc.tensor.matmul(
                    pwarm[:, (w % 8) * 128:(w % 8 + 1) * 128],
                    warm_sb[:],
                    warm_sb[:],
                    start=True,
                    stop=True,
                )

            for o in range(n_tiles):
                # one 1-MiB input DMA, 8 KiB contiguous per partition; the
                # first two tiles stream in as quarter-DMAs so group (0,0)'s
                # matmuls start after 256 KiB instead of 1 MiB.
                xh = xh_pool.tile([128, 4096], F16)
                if o < 2:
                    for q in range(4):
                        sl = slice(q * 1024, (q + 1) * 1024)
                        nc.sync.dma_start(out=xh[:, sl], in_=xv[o][:, sl])
                else:
                    nc.sync.dma_start(out=xh[:], in_=xv[o])

                yt = y_pool.tile([128, 4096], F16)
                for c in range(2):
                    for h in range(2):
                        # stage 1: W[p=(g,j), f=(mp8,a2,k)] for 32 tokens
                        pw = pw_pool.tile([128, 1024], F32)
                        for mp8 in range(8):
                            mp = h * 8 + mp8
                            nc.tensor.matmul(
                                pw[:, mp8 * 128:(mp8 + 1) * 128],
                                xh[:, c * 2048 + mp * 128:
                                   c * 2048 + (mp + 1) * 128],
                                uu_sb[:],
                                start=True,
                                stop=True,
                            )
                        wt = wt_pool.tile([128, 1024], F16)
                        nc.vector.tensor_copy(out=wt[:], in_=pw[:])

                        # stage 2: Y[p=(g,l), f=(s,mp4,a2,k)], VV stationary;
                        # one PSUM bank per matmul so ACT evacuates each half
                        # as soon as its matmul drains.  Four 1-bank py slots
                        # (not two 2-bank ones) -- the extra pipeline depth is
                        # worth more than the per-op overhead of FD=512 reads.
                        for s in range(2):
                            py = py_pool.tile([128, 512], F32)
                            nc.tensor.matmul(
                                py[:],
                                vv_sb[:],
                                wt[:, s * 512:(s + 1) * 512],
                                start=True,
                                stop=True,
                            )
                            off = (c * 2 + h) * 1024 + s * 512
                            # ~6% of Y evacuations go to DVE to balance the
                            # ACT/DVE totals (ACT otherwise runs ~90us vs
                            # DVE ~78us per core).
                            if o % 2 == 1 and c == 0 and h == 0 and s == 0:
                                nc.vector.tensor_copy(
                                    out=yt[:, off:off + 512], in_=py[:]
                                )
                            else:
                                nc.scalar.copy(out=yt[:, off:off + 512], in_=py[:])
                # output goes out on the SWDGE ring from the otherwise-idle
                # GpSimd engine: the SP HWDGE ring keeps the input stream,
                # and ACT does evacuation only.  The last two tiles go out
                # as per-group quarter-DMAs (alternating rings) so the tail
                # bytes hit HBM right after each compute group instead of
                # waiting for the whole tile.
                yvo = yv[o]
                if o >= n_tiles - 2:
                    for q in range(4):
                        sl = slice(q * 1024, (q + 1) * 1024)
                        eng = nc.gpsimd if q % 2 == 0 else nc.scalar
                        eng.dma_start(out=yvo[:, sl], in_=yt[:, sl])
                else:
                    nc.gpsimd.dma_start(out=yvo, in_=yt[:])

    nc.finalize()
    return nc


def _get_nc() -> bass.Bass:
    if "nc" not in _CACHE:
        _CACHE["nc"] = _build_nc()
    return _CACHE["nc"]


def kernel(x: np.ndarray, U: np.ndarray, V: np.ndarray, b: np.ndarray) -> np.ndarray:
    global LAST_RESULTS
    assert x.shape == (B, S, D) and U.shape == (64, 64) and V.shape == (64, 64)

    nc = _get_nc()

    eye2 = np.eye(2, dtype=np.float32)
    uu = np.kron(eye2, np.asarray(U, dtype=np.float32)).astype(np.float16)
    vv = np.kron(eye2, np.asarray(V, dtype=np.float32)).astype(np.float16)

    xf = np.asarray(x, dtype=np.float32).reshape(T, D)
    in_maps = []
    for c in range(N_CORES):
        # token tau = o*128 + c2*64 + 2*m + a2, d = 64*i + j:
        # axes (o, c2, m, a2, i, j) -> [p=(a2,i), f=(o, c2, m, j)]
        xdev = (
            xf[c * TPC:(c + 1) * TPC]
            .reshape(N_TILES, 2, 32, 2, 64, 64)
            .astype(np.float16)
            .transpose(3, 4, 0, 1, 2, 5)
            .reshape(128, N_TILES * 4096)
        )
        in_maps.append({"x": np.ascontiguousarray(xdev), "uu": uu, "vv": vv})

    res = run_bass_kernel_spmd(
        nc,
        in_maps,
        core_ids=list(range(N_CORES)),
        trace=bool(os.environ.get("BASS_TRACE")),
    )
    LAST_RESULTS = res

    out = np.empty((T, D), dtype=np.float32)
    for c in range(N_CORES):
        # y_dev axes (g, l | o | c2, h, s, mp4, a2, k)
        #   token tau = o*128 + c2*64 + 32*h + 16*s + 4*mp4 + 2*g + a2
        #   d' = 64*k + l
        ydev = res.results[c]["y"].reshape(2, 64, N_TILES, 2, 2, 2, 4, 2, 64)
        out[c * TPC:(c + 1) * TPC] = (
            ydev.transpose(2, 3, 4, 5, 6, 0, 7, 8, 1).reshape(TPC, D)
        )

    if np.any(np.asarray(b) != 0):
        out += np.asarray(b, dtype=np.float32)
    return out.reshape(B, S, D)


# revision 29
# speedup vs baseline: 1.0595x; 1.0022x over previous
"""CoLA linear kernel for Trainium2: y = x @ kron(U, V) + b.

Math: per token t (16384 of them), with X_t = x[t].reshape(64, 64),
    y[t] = flatten(U^T @ X_t @ V) + b     (row-major flatten, d' = 64*k + l)

Distribution: pure data parallel over tokens, 2048 per NeuronCore x 8 cores;
U, V are tiny and replicated; b is added on the host (zeros in practice).

The kernel is memory-bound, so the layout minimizes HBM bytes and DMA
descriptor overhead; the steady state is paced by PSUM evacuation:

  - fp16 device I/O: x is converted fp16 and pre-permuted on the host into
    the exact SBUF layout the matmuls need; y is written fp16 in the PSUM
    layout and un-permuted on the host.  Device HBM traffic halves to
    16 MiB in + 16 MiB out per core, and every DMA is [p=128, 8 KiB
    contiguous per partition] -- maximal descriptors, no on-chip permutes.
    Measured combined r+w DMA rate mid-run: ~425 GB/s per core.
  - x SBUF tile (128 tokens): [p=(a2,i), f=(c,m,j)], token = o*128 + c*64
    + 2*m + a2, d = 64*i + j.
  - Stage 1 (contract i): per m-pair mp, lhsT = x slice [p=(a2,i), f=(g,j)]
    (stationary, FWL since fp16 128-col), rhs = UU = kron(I2, U) ->
    W [p=(g,j), f=(mp8,a2,k)] in a 2-bank PSUM tile (8 matmuls, N=128).
  - DVE evacuates W to SBUF fp16 (one FD=1024 copy per 2-bank tile).
  - Stage 2 (contract j): lhsT = VV = kron(I2, V) (stationary const),
    rhs = wt halves moving (N=512) -> Y [p=(g,l), f=(s,mp4,a2,k)] in
    1-bank PSUM tiles (4 slots: pipeline depth beats per-op overhead).
  - ACT evacuates Y to SBUF fp16 (plus ~6% on DVE to balance engine
    totals); output DMAs ride the SWDGE ring from the idle GpSimd engine.
  - PSUM->SBUF evacuation (1x mode, two engines) is the steady-state
    pace at ~5.2us/128-token tile; a 24-matmul warmup burst lifts the PE
    HAM clock-gate before the real stream, and the first/last tiles use
    quarter-DMAs to shorten pipeline ramp and drain.
  - fp32 PSUM accumulation throughout; measured rel err ~4.7e-4.
"""

import os

import numpy as np

import concourse.bacc as bacc
import concourse.bass as bass
import concourse.mybir as mybir
import concourse.tile as tile
from concourse.bass_utils import run_bass_kernel_spmd

N_CORES = 8
B, S, D = 4, 4096, 4096
T = B * S                  # 16384 tokens
TPC = T // N_CORES         # 2048 tokens per core
TOK_PER_TILE = 128         # tokens handled per DMA tile
N_TILES = TPC // TOK_PER_TILE  # 16

F32 = mybir.dt.float32
F16 = mybir.dt.float16

LAST_RESULTS = None        # test harness can inspect exec_time_ns etc.

_CACHE: dict = {}


def _build_nc(tpc: int = TPC) -> bass.Bass:
    n_tiles = tpc // TOK_PER_TILE
    nc = bacc.Bacc()

    x = nc.dram_tensor("x", [128, n_tiles * 4096], F16, kind="ExternalInput")
    uu = nc.dram_tensor("uu", [128, 128], F16, kind="ExternalInput")
    vv = nc.dram_tensor("vv", [128, 128], F16, kind="ExternalInput")
    y = nc.dram_tensor("y", [128, n_tiles * 4096], F16, kind="ExternalOutput")

    xv = x[:].rearrange("p (o f) -> o p f", f=4096)
    yv = y[:].rearrange("p (o f) -> o p f", f=4096)

    with tile.TileContext(nc) as tc:
        with (
            tc.tile_pool(name="consts", bufs=1) as cpool,
            tc.tile_pool(name="xh", bufs=6) as xh_pool,
            tc.tile_pool(name="wt", bufs=6) as wt_pool,
            tc.tile_pool(name="yo", bufs=4) as y_pool,
            tc.tile_pool(name="pw", bufs=2, space="PSUM") as pw_pool,
            tc.tile_pool(name="py", bufs=4, space="PSUM") as py_pool,
        ):
            # consts go on the ACT ring so the Sync engine's first issue is
            # the tile-0 input DMA (each HWDGE issue costs ~0.7us of NX time
            # and they serialize per engine).
            uu_sb = cpool.tile([128, 128], F16)
            nc.scalar.dma_start(out=uu_sb[:], in_=uu[:])
            vv_sb = cpool.tile([128, 128], F16)
            nc.scalar.dma_start(out=vv_sb[:], in_=vv[:])

            # ~2.5us of dummy matmuls on an uninitialized tile (no DMA
            # dependency, so they start right after the entry barrier): keeps
            # the PE busy from t~6.5us so the HAM clock-gate lifts to 2.4 GHz
            # by the time the real matmul stream is underway.
            warm_sb = cpool.tile([128, 128], F16)
            nc.gpsimd.memset(warm_sb[:], 0.0)
            pwarm = pw_pool.tile([128, 1024], F32, tag="pw")
            for w in range(24):
                nc.tensor.matmul(
                    pwarm[:, (w % 8) * 128:(w % 8 + 1) * 128],
                    warm_sb[:],
                    warm_sb[:],
                    start=True,
                    stop=True,
                )

            for o in range(n_tiles):
                # one 1-MiB input DMA, 8 KiB contiguous per partition; the
                # first two tiles stream in as quarter-DMAs so group (0,0)'s
                # matmuls start after 256 KiB instead of 1 MiB.
                xh = xh_pool.tile([128, 4096], F16)
                if o < 2:
                    for q in range(4):
                        sl = slice(q * 1024, (q + 1) * 1024)
                        nc.sync.dma_start(out=xh[:, sl], in_=xv[o][:, sl])
                else:
                    nc.sync.dma_start(out=xh[:], in_=xv[o])

                yt = y_pool.tile([128, 4096], F16)
                for c in range(2):
                    for h in range(2):
                        # stage 1: W[p=(g,j), f=(mp8,a2,k)] for 32 tokens
                        pw = pw_pool.tile([128, 1024], F32)
                        for mp8 in range(8):
                            mp = h * 8 + mp8
                            nc.tensor.matmul(
                                pw[:, mp8 * 128:(mp8 + 1) * 128],
                                xh[:, c * 2048 + mp * 128:
                                   c * 2048 + (mp + 1) * 128],
                                uu_sb[:],
                                start=True,
                                stop=True,
                            )
                        wt = wt_pool.tile([128, 1024], F16)
                        nc.vector.tensor_copy(out=wt[:], in_=pw[:])

                        # stage 2: Y[p=(g,l), f=(s,mp4,a2,k)], VV stationary;
                        # one PSUM bank per matmul so ACT evacuates each half
                        # as soon as its matmul drains.  Four 1-bank py slots
                        # (not two 2-bank ones) -- the extra pipeline depth is
                        # worth more than the per-op overhead of FD=512 reads.
                        for s in range(2):
                            py = py_pool.tile([128, 512], F32)
                            nc.tensor.matmul(
                                py[:],
                                vv_sb[:],
                                wt[:, s * 512:(s + 1) * 512],
                                start=True,
                                stop=True,
                            )
                            off = (c * 2 + h) * 1024 + s * 512
                            # ~6% of Y evacuations go to DVE to balance the
                            # ACT/DVE totals (ACT otherwise runs ~90us vs
                            # DVE ~78us per core).
                            if o % 2 == 1 and c == 0 and h == 0 and s == 0:
                                nc.vector.tensor_copy(
                                    out=yt[:, off:off + 512], in_=py[:]
                                )
                            else:
                                nc.scalar.copy(out=yt[:, off:off + 512], in_=py[:])
                # output goes out on the SWDGE ring from the otherwise-idle
                # GpSimd engine: the SP HWDGE ring keeps the input stream,
                # and ACT does evacuation only.  The last two tiles go out
                # as per-group quarter-DMAs (alternating rings) so the tail
                # bytes hit HBM right after each compute group instead of
                # waiting for the whole tile.
                yvo = yv[o]
                if o >= n_tiles - 2:
                    for q in range(4):
                        sl = slice(q * 1024, (q + 1) * 1024)
                        eng = nc.gpsimd if q % 2 == 0 else nc.scalar
                        eng.dma_start(out=yvo[:, sl], in_=yt[:, sl])
                else:
                    nc.gpsimd.dma_start(out=yvo, in_=yt[:])

    nc.finalize()
    return nc


def _get_nc() -> bass.Bass:
    if "nc" not in _CACHE:
        _CACHE["nc"] = _build_nc()
    return _CACHE["nc"]


def kernel(x: np.ndarray, U: np.ndarray, V: np.ndarray, b: np.ndarray) -> np.ndarray:
    global LAST_RESULTS
    assert x.shape == (B, S, D) and U.shape == (64, 64) and V.shape == (64, 64)

    nc = _get_nc()

    eye2 = np.eye(2, dtype=np.float32)
    uu = np.kron(eye2, np.asarray(U, dtype=np.float32)).astype(np.float16)
    vv = np.kron(eye2, np.asarray(V, dtype=np.float32)).astype(np.float16)

    xf = np.asarray(x, dtype=np.float32).reshape(T, D)
    in_maps = []
    for c in range(N_CORES):
        # token tau = o*128 + c2*64 + 2*m + a2, d = 64*i + j:
        # axes (o, c2, m, a2, i, j) -> [p=(a2,i), f=(o, c2, m, j)]
        xdev = (
            xf[c * TPC:(c + 1) * TPC]
            .reshape(N_TILES, 2, 32, 2, 64, 64)
            .astype(np.float16)
            .transpose(3, 4, 0, 1, 2, 5)
            .reshape(128, N_TILES * 4096)
        )
        in_maps.append({"x": np.ascontiguousarray(xdev), "uu": uu, "vv": vv})

    res = run_bass_kernel_spmd(
        nc,
        in_maps,
        core_ids=list(range(N_CORES)),
        trace=bool(os.environ.get("BASS_TRACE")),
    )
    LAST_RESULTS = res

    out = np.empty((T, D), dtype=np.float32)
    for c in range(N_CORES):
        # y_dev axes (g, l | o | c2, h, s, mp4, a2, k)
        #   token tau = o*128 + c2*64 + 32*h + 16*s + 4*mp4 + 2*g + a2
        #   d' = 64*k + l
        ydev = res.results[c]["y"].reshape(2, 64, N_TILES, 2, 2, 2, 4, 2, 64)
        out[c * TPC:(c + 1) * TPC] = (
            ydev.transpose(2, 3, 4, 5, 6, 0, 7, 8, 1).reshape(TPC, D)
        )

    if np.any(np.asarray(b) != 0):
        out += np.asarray(b, dtype=np.float32)
    return out.reshape(B, S, D)
